# revision 33
# baseline (speedup 1.0000x reference)
"""Trainium2 Bass kernel for nn_DotAtt_40097814675537.

Math (matches the reference exactly up to fp rounding):
    score = Q @ K^T / sqrt(d)        [B, Sq, Sk]
    x     = score @ V                [B, Sq, dv]
    out   = softmax(where(j > valid_len[q], -1e6, x[b, q, j]), axis=-1)

Optimizations:
  * Associativity: x = (Q / sqrt(d)) @ (K^T @ V) - 4x fewer FLOPs
    (contraction 2048 -> 512 for the big matmul; no nonlinearity between
    the two matmuls so this is exact math, only fp rounding differs).
  * Data-parallel over batch B=8, one batch per NeuronCore, no collectives.
  * fp32-accurate matmuls from 3 float16 passes (hi/lo split): each fp32
    operand x = hi + lo with hi=fp16(x), lo=fp16(x-hi), then
    a@b ~= ah@bh + ah@bl + al@bh (al@bl ~ 2^-22 rel, dropped).  Runs at
    the fp16 PE rate: 3 cyc/row total vs 4 cyc/row for native fp32.
  * Sorted-query specialization: rows whose mask kills column j produce
    EXACTLY 0 in the output (exp underflows), so for each 128-row tile only
    columns [0, max(valid_len)+1) need computing.  The host sorts queries
    by valid_len (softmax is row-wise, so a row permutation is exact) and
    the kernel computes a per-tile column width; unwritten output stays 0
    (output buffers are pre-zeroed).  Host inverse-permutes the result.
    The build is cached per width-tuple, so any input data is handled
    correctly (seed-dependent widths just trigger a rebuild).
"""

import math
import sys
import types

import numpy as np

B, SQ, SK, D, DV = 8, 2048, 2048, 512, 512
N_CORES = 8
P = 128  # partitions
SC = SK // P  # 16 s-chunks for the K^T V contraction
DC = D // P  # 4 d-chunks for the Q M contraction
QT_TILES = SQ // P  # 16 query row tiles
NEG_FILL = -1000000.0

_CACHE = {}


def _install_ntff_hook():
    """antenv.axon_hooks is absent in this image; provide it so trace=True
    profiling works when requested (used by test.py, harmless otherwise)."""
    if "antenv.axon_hooks" in sys.modules:
        return
    try:
        from trn_agent_boot.trn_boot import _ntff_profile_via_ctypes

        hook = _ntff_profile_via_ctypes("/opt/axon/libaxon_pjrt.so")
    except Exception:
        hook = None
    mod = types.ModuleType("antenv.axon_hooks")
    mod.get_axon_ntff_profile_hook = lambda: hook
    mod.set_axon_ntff_profile_hook = lambda h: None
    sys.modules["antenv.axon_hooks"] = mod


def _build(widths):
    import concourse.tile as tile
    from concourse import bacc, mybir

    nc = bacc.Bacc("TRN2", target_bir_lowering=False, debug=False, num_devices=N_CORES)
    f32 = mybir.dt.float32
    f16 = mybir.dt.float16
    bf16 = mybir.dt.bfloat16

    sum_w = sum(widths)
    offs = [0]
    for w in widths:
        offs.append(offs[-1] + w)

    # All big inputs arrive PARTITION-MAJOR so each SBUF partition's data is
    # one contiguous DRAM run (8KB DMA descriptors instead of 2KB rows; the
    # DMA queues are descriptor-rate-bound otherwise).  Layouts (fp16):
    #   k:  [128, SC*2*D]  k[p, s*1024 + :]   = packed row  s*128+p  of K
    #   v:  [128, SC*2*DV] v[p, s*1024 + :]   = packed row  s*128+p  of V
    #   qt: [128, DC*2*SQ] qt[p, c*4096 + :]  = packed row  c*128+p  of Q^T
    # Each is loaded in NBLK column blocks so compute can start early.
    NBLK = 8
    KCOLS, QCOLS = SC * 2 * D, DC * 2 * SQ
    k_d = nc.dram_tensor("k", [P, KCOLS], f16, kind="ExternalInput")
    v_d = nc.dram_tensor("v", [P, KCOLS], f16, kind="ExternalInput")
    qt_d = nc.dram_tensor("qt", [P, QCOLS], f16, kind="ExternalInput")
    mask_d = nc.dram_tensor("mask", [P, sum_w], bf16, kind="ExternalInput")
    o_d = nc.dram_tensor("o", [SQ, DV], f32, kind="ExternalOutput")

    with tile.TileContext(nc) as tc:
        with (
            tc.tile_pool(name="consts", bufs=1) as consts,
            tc.tile_pool(name="big", bufs=1) as big,
            tc.tile_pool(name="mprime", bufs=1) as mp_pool,
            tc.tile_pool(name="psm", bufs=1, space="PSUM") as psum_m,
            tc.tile_pool(name="psx", bufs=4, space="PSUM") as psum_x,
            tc.tile_pool(name="work", bufs=3) as work,
            tc.tile_pool(name="stats", bufs=4) as stats,
        ):
            mask_t = consts.tile([P, sum_w], bf16, tag="mask")
            kt = big.tile([P, KCOLS], f16, tag="k", name="k_sb")
            vt = big.tile([P, KCOLS], f16, tag="v", name="v_sb")
            qtt = big.tile([P, QCOLS], f16, tag="qt", name="qt_sb")

            # one sequencer issues every load in priority order: the DMA
            # queues drain roughly in issue order, so K/V blocks beat QT/mask
            kb, qb = KCOLS // NBLK, QCOLS // NBLK
            for blk in range(NBLK):
                nc.sync.dma_start(
                    out=kt[:, blk * kb : (blk + 1) * kb],
                    in_=k_d[:, blk * kb : (blk + 1) * kb],
                )
                nc.sync.dma_start(
                    out=vt[:, blk * kb : (blk + 1) * kb],
                    in_=v_d[:, blk * kb : (blk + 1) * kb],
                )
            for blk in range(NBLK):
                nc.sync.dma_start(
                    out=qtt[:, blk * qb : (blk + 1) * qb],
                    in_=qt_d[:, blk * qb : (blk + 1) * qb],
                )
            nc.sync.dma_start(out=mask_t, in_=mask_d[:, :])

            # Phase 1: M = K^T V over 16 s-chunks, 3 fp16 passes each
            psums = [
                psum_m.tile([P, DV], f32, tag=f"m{c}", name=f"psum_m{c}")
                for c in range(DC)
            ]
            for s in range(SC):
                base = s * 2 * D
                vh = vt[:, base : base + DV]
                vlo = vt[:, base + DV : base + 2 * DV]
                for c in range(DC):
                    kh = kt[:, base + c * P : base + (c + 1) * P]
                    klo = kt[:, base + D + c * P : base + D + (c + 1) * P]
                    # same-weight passes adjacent to reuse loaded weights
                    nc.tensor.matmul(
                        psums[c][:, :], kh, vh, start=(s == 0), stop=False
                    )
                    nc.tensor.matmul(psums[c][:, :], kh, vlo, start=False, stop=False)
                    nc.tensor.matmul(
                        psums[c][:, :], klo, vh, start=False, stop=(s == SC - 1)
                    )

            # M PSUM -> SBUF split into fp16 hi/lo (ScalarE cast + DVE residual)
            mhis, mlos = [], []
            for c in range(DC):
                mhi = mp_pool.tile([P, DV], f16, tag=f"mh{c}", name=f"mhi{c}")
                nc.scalar.copy(mhi[:, :], psums[c][:, :])
                mlo = mp_pool.tile([P, DV], f16, tag=f"ml{c}", name=f"mlo{c}")
                nc.vector.tensor_sub(mlo[:, :], psums[c][:, :], mhi[:, :])
                mhis.append(mhi)
                mlos.append(mlo)

            # Phase 2: per query tile (width W): X = Q M, mask, softmax, store.
            # Widest tiles first so the last tile's softmax tail is shortest.
            order = sorted(range(QT_TILES), key=lambda i: widths[i], reverse=True)
            for ti, t in enumerate(order):
                W = widths[t]
                px = psum_x.tile([P, DV], f32, tag="x")
                for c in range(DC):
                    qh = qtt[:, c * 2 * SQ + t * P : c * 2 * SQ + (t + 1) * P]
                    qlo = qtt[
                        :, c * 2 * SQ + SQ + t * P : c * 2 * SQ + SQ + (t + 1) * P
                    ]
                    nc.tensor.matmul(
                        px[:, 0:W], qh, mhis[c][:, 0:W], start=(c == 0), stop=False
                    )
                    nc.tensor.matmul(
                        px[:, 0:W], qh, mlos[c][:, 0:W], start=False, stop=False
                    )
                    nc.tensor.matmul(
                        px[:, 0:W],
                        qlo,
                        mhis[c][:, 0:W],
                        start=False,
                        stop=(c == DC - 1),
                    )
                xs = work.tile([P, DV], f32, tag="x")
                nc.vector.tensor_add(
                    xs[:, 0:W], px[:, 0:W], mask_t[:, offs[t] : offs[t] + W]
                )
                nmx = stats.tile([P, 1], f32, tag="nmx")
                nc.vector.tensor_reduce(
                    out=nmx,
                    in_=xs[:, 0:W],
                    axis=mybir.AxisListType.X,
                    op=mybir.AluOpType.max,
                    negate=True,
                )
                ex = work.tile([P, DV], f32, tag="e")
                sm = stats.tile([P, 1], f32, tag="sum")
                nc.scalar.activation(
                    ex[:, 0:W],
                    xs[:, 0:W],
                    mybir.ActivationFunctionType.Exp,
                    bias=nmx[:, :],
                    scale=1.0,
                    accum_out=sm[:, :],
                )
                rs = stats.tile([P, 1], f32, tag="r")
                nc.vector.reciprocal(rs, sm)
                ot = work.tile([P, DV], f32, tag="o")
                # normalize alternately on ScalarE/VectorE to balance both;
                # store from SyncE, which is idle during phase 2
                if ti % 2 == 0:
                    nc.scalar.mul(ot[:, 0:W], ex[:, 0:W], rs[:, :])
                else:
                    nc.vector.tensor_scalar_mul(ot[:, 0:W], ex[:, 0:W], rs[:, :])
                nc.sync.dma_start(
                    out=o_d[t * P : (t + 1) * P, 0:W], in_=ot[:, 0:W]
                )

    nc.compile()
    return nc


def _split16_pack(x):
    """[..., n] fp32 -> [..., 2n] fp16 packed [hi | lo] along the last axis."""
    hi = x.astype(np.float16)
    lo = (x - hi.astype(np.float32)).astype(np.float16)
    return np.ascontiguousarray(np.concatenate([hi, lo], axis=-1))


def _part_major(x):
    """[G*128, C] -> [128, G*C]: partition p holds rows p, 128+p, ... so each
    SBUF partition's data is one contiguous DRAM run per block."""
    g = x.shape[0] // P
    return np.ascontiguousarray(
        x.reshape(g, P, x.shape[1]).transpose(1, 0, 2).reshape(P, -1)
    )


def _get_nc(widths):
    key = tuple(widths)
    if key not in _CACHE:
        _install_ntff_hook()
        _CACHE[key] = _build(key)
    return _CACHE[key]


def kernel(K, V, Q, valid_len, _trace=False):
    import ml_dtypes

    from concourse.bass_utils import run_bass_kernel_spmd

    K = np.ascontiguousarray(np.asarray(K, dtype=np.float32))
    V = np.ascontiguousarray(np.asarray(V, dtype=np.float32))
    Q = np.asarray(Q, dtype=np.float32)
    vl = np.asarray(valid_len).astype(np.int64)

    # sort queries by valid_len (row permutation; exact for row-wise softmax)
    perm = np.argsort(vl, kind="stable")
    vls = vl[perm]
    widths = []
    for t in range(QT_TILES):
        w = int(vls[t * P : (t + 1) * P].max()) + 1
        widths.append(min(DV, -(-w // 32) * 32))
    widths = tuple(widths)
    sum_w = sum(widths)

    # Q^T per batch: permuted rows, pre-scaled by 1/sqrt(d), fp16 hi/lo packed
    scale = np.float32(1.0 / math.sqrt(D))
    qp = Q[:, perm, :] * scale
    qt = np.ascontiguousarray(qp.transpose(0, 2, 1))

    # additive mask for the sorted rows, packed per tile: [128, sum_w] bf16
    # (bf16 is exact here: values are only 0 / -1e6-ish; masked lanes
    # underflow to 0 after exp either way, identical to masked_fill)
    col = np.arange(DV, dtype=np.int64)
    mask_full = np.where(
        col[None, :] > vls[:, None], np.float32(NEG_FILL), np.float32(0.0)
    )
    mask_packed = np.empty((P, sum_w), dtype=ml_dtypes.bfloat16)
    off = 0
    for t in range(QT_TILES):
        w = widths[t]
        mask_packed[:, off : off + w] = mask_full[t * P : (t + 1) * P, :w].astype(
            ml_dtypes.bfloat16
        )
        off += w

    nc = _get_nc(widths)
    in_maps = [
        {
            "k": _part_major(_split16_pack(K[b])),
            "v": _part_major(_split16_pack(V[b])),
            "qt": _part_major(_split16_pack(qt[b])),
            "mask": mask_packed,
        }
        for b in range(N_CORES)
    ]
    res = run_bass_kernel_spmd(
        nc, in_maps, core_ids=list(range(N_CORES)), trace=_trace
    )
    # device rows r correspond to original queries perm[r]; unwritten
    # (masked) columns stay 0 from the pre-zeroed output buffers
    out = np.empty((B, SQ, DV), dtype=np.float32)
    for b in range(N_CORES):
        out[b, perm, :] = res.results[b]["o"]
    if _trace:
        kernel.last_result = res
    return out


# revision 36
# speedup vs baseline: 1.0414x; 1.0414x over previous
"""Trainium2 Bass kernel for nn_DotAtt_40097814675537.

Math (matches the reference exactly up to fp rounding):
    score = Q @ K^T / sqrt(d)        [B, Sq, Sk]
    x     = score @ V                [B, Sq, dv]
    out   = softmax(where(j > valid_len[q], -1e6, x[b, q, j]), axis=-1)

Optimizations:
  * Associativity: x = (Q / sqrt(d)) @ (K^T @ V) - 4x fewer FLOPs
    (contraction 2048 -> 512 for the big matmul; no nonlinearity between
    the two matmuls so this is exact math, only fp rounding differs).
  * Data-parallel over batch B=8, one batch per NeuronCore, no collectives.
  * fp32-accurate matmuls from 3 float16 passes (hi/lo split): each fp32
    operand x = hi + lo with hi=fp16(x), lo=fp16(x-hi), then
    a@b ~= ah@bh + ah@bl + al@bh (al@bl ~ 2^-22 rel, dropped).  Runs at
    the fp16 PE rate: 3 cyc/row total vs 4 cyc/row for native fp32.
  * Sorted-query specialization: rows whose mask kills column j produce
    EXACTLY 0 in the output (exp underflows), so for each 128-row tile only
    columns [0, max(valid_len)+1) need computing.  The host sorts queries
    by valid_len (softmax is row-wise, so a row permutation is exact) and
    the kernel computes a per-tile column width; unwritten output stays 0
    (output buffers are pre-zeroed).  Host inverse-permutes the result.
    The build is cached per width-tuple, so any input data is handled
    correctly (seed-dependent widths just trigger a rebuild).
"""

import math
import sys
import types

import numpy as np

B, SQ, SK, D, DV = 8, 2048, 2048, 512, 512
N_CORES = 8
P = 128  # partitions
SC = SK // P  # 16 s-chunks for the K^T V contraction
DC = D // P  # 4 d-chunks for the Q M contraction
QT_TILES = SQ // P  # 16 query row tiles
NEG_FILL = -1000000.0

_CACHE = {}


def _install_ntff_hook():
    """antenv.axon_hooks is absent in this image; provide it so trace=True
    profiling works when requested (used by test.py, harmless otherwise)."""
    if "antenv.axon_hooks" in sys.modules:
        return
    try:
        from trn_agent_boot.trn_boot import _ntff_profile_via_ctypes

        hook = _ntff_profile_via_ctypes("/opt/axon/libaxon_pjrt.so")
    except Exception:
        hook = None
    mod = types.ModuleType("antenv.axon_hooks")
    mod.get_axon_ntff_profile_hook = lambda: hook
    mod.set_axon_ntff_profile_hook = lambda h: None
    sys.modules["antenv.axon_hooks"] = mod


def _build(widths):
    import concourse.tile as tile
    from concourse import bacc, mybir

    nc = bacc.Bacc("TRN2", target_bir_lowering=False, debug=False, num_devices=N_CORES)
    f32 = mybir.dt.float32
    f16 = mybir.dt.float16
    bf16 = mybir.dt.bfloat16

    sum_w = sum(widths)
    offs = [0]
    for w in widths:
        offs.append(offs[-1] + w)

    # All big inputs arrive PARTITION-MAJOR so each SBUF partition's data is
    # one contiguous DRAM run (8KB DMA descriptors instead of 2KB rows; the
    # DMA queues are descriptor-rate-bound otherwise).  Layouts (fp16):
    #   k:  [128, SC*2*D]  k[p, s*1024 + :]   = packed row  s*128+p  of K
    #   v:  [128, SC*2*DV] v[p, s*1024 + :]   = packed row  s*128+p  of V
    #   qt: [128, DC*2*SQ] qt[p, c*4096 + :]  = packed row  c*128+p  of Q^T
    # Each is loaded in NBLK column blocks so compute can start early.
    NBLK = 8
    KCOLS, QCOLS = SC * 2 * D, DC * 2 * SQ
    k_d = nc.dram_tensor("k", [P, KCOLS], f16, kind="ExternalInput")
    v_d = nc.dram_tensor("v", [P, KCOLS], f16, kind="ExternalInput")
    qt_d = nc.dram_tensor("qt", [P, QCOLS], f16, kind="ExternalInput")
    mask_d = nc.dram_tensor("mask", [P, sum_w], bf16, kind="ExternalInput")
    o_d = nc.dram_tensor("o", [SQ, DV], f32, kind="ExternalOutput")

    with tile.TileContext(nc) as tc:
        with (
            tc.tile_pool(name="consts", bufs=1) as consts,
            tc.tile_pool(name="big", bufs=1) as big,
            tc.tile_pool(name="mprime", bufs=1) as mp_pool,
            tc.tile_pool(name="psm", bufs=1, space="PSUM") as psum_m,
            tc.tile_pool(name="psx", bufs=4, space="PSUM") as psum_x,
            tc.tile_pool(name="work", bufs=3) as work,
            tc.tile_pool(name="stats", bufs=4) as stats,
        ):
            mask_t = consts.tile([P, sum_w], bf16, tag="mask")
            kt = big.tile([P, KCOLS], f16, tag="k", name="k_sb")
            vt = big.tile([P, KCOLS], f16, tag="v", name="v_sb")
            qtt = big.tile([P, QCOLS], f16, tag="qt", name="qt_sb")

            # one sequencer issues every load in priority order: the DMA
            # queues drain roughly in issue order, so K/V blocks beat QT/mask.
            # Geometric block sizes (in s-chunks) so the first matmuls start
            # as early as possible while later blocks amortize issue cost.
            CHUNK = 2 * D  # columns per s-chunk
            kblocks = [1, 1, 2, 4, 8]
            pos = 0
            for nchunk in kblocks:
                lo, hi = pos * CHUNK, (pos + nchunk) * CHUNK
                nc.sync.dma_start(out=kt[:, lo:hi], in_=k_d[:, lo:hi])
                nc.sync.dma_start(out=vt[:, lo:hi], in_=v_d[:, lo:hi])
                pos += nchunk
            qb = QCOLS // NBLK
            for blk in range(NBLK):
                nc.sync.dma_start(
                    out=qtt[:, blk * qb : (blk + 1) * qb],
                    in_=qt_d[:, blk * qb : (blk + 1) * qb],
                )
            nc.sync.dma_start(out=mask_t, in_=mask_d[:, :])

            # Phase 1: M = K^T V over 16 s-chunks, 3 fp16 passes each
            psums = [
                psum_m.tile([P, DV], f32, tag=f"m{c}", name=f"psum_m{c}")
                for c in range(DC)
            ]
            for s in range(SC):
                base = s * 2 * D
                vh = vt[:, base : base + DV]
                vlo = vt[:, base + DV : base + 2 * DV]
                for c in range(DC):
                    kh = kt[:, base + c * P : base + (c + 1) * P]
                    klo = kt[:, base + D + c * P : base + D + (c + 1) * P]
                    # same-weight passes adjacent to reuse loaded weights
                    nc.tensor.matmul(
                        psums[c][:, :], kh, vh, start=(s == 0), stop=False
                    )
                    nc.tensor.matmul(psums[c][:, :], kh, vlo, start=False, stop=False)
                    nc.tensor.matmul(
                        psums[c][:, :], klo, vh, start=False, stop=(s == SC - 1)
                    )

            # M PSUM -> SBUF split into fp16 hi/lo (ScalarE cast + DVE residual)
            mhis, mlos = [], []
            for c in range(DC):
                mhi = mp_pool.tile([P, DV], f16, tag=f"mh{c}", name=f"mhi{c}")
                nc.scalar.copy(mhi[:, :], psums[c][:, :])
                mlo = mp_pool.tile([P, DV], f16, tag=f"ml{c}", name=f"mlo{c}")
                nc.vector.tensor_sub(mlo[:, :], psums[c][:, :], mhi[:, :])
                mhis.append(mhi)
                mlos.append(mlo)

            # Phase 2: per query tile (width W): X = Q M, mask, softmax, store.
            # Widest tiles first so the last tile's softmax tail is shortest.
            order = sorted(range(QT_TILES), key=lambda i: widths[i], reverse=True)
            for ti, t in enumerate(order):
                W = widths[t]
                px = psum_x.tile([P, DV], f32, tag="x")
                for c in range(DC):
                    qh = qtt[:, c * 2 * SQ + t * P : c * 2 * SQ + (t + 1) * P]
                    qlo = qtt[
                        :, c * 2 * SQ + SQ + t * P : c * 2 * SQ + SQ + (t + 1) * P
                    ]
                    nc.tensor.matmul(
                        px[:, 0:W], qh, mhis[c][:, 0:W], start=(c == 0), stop=False
                    )
                    nc.tensor.matmul(
                        px[:, 0:W], qh, mlos[c][:, 0:W], start=False, stop=False
                    )
                    nc.tensor.matmul(
                        px[:, 0:W],
                        qlo,
                        mhis[c][:, 0:W],
                        start=False,
                        stop=(c == DC - 1),
                    )
                xs = work.tile([P, DV], f32, tag="x")
                nc.vector.tensor_add(
                    xs[:, 0:W], px[:, 0:W], mask_t[:, offs[t] : offs[t] + W]
                )
                nmx = stats.tile([P, 1], f32, tag="nmx")
                nc.vector.tensor_reduce(
                    out=nmx,
                    in_=xs[:, 0:W],
                    axis=mybir.AxisListType.X,
                    op=mybir.AluOpType.max,
                    negate=True,
                )
                ex = work.tile([P, DV], f32, tag="e")
                sm = stats.tile([P, 1], f32, tag="sum")
                nc.scalar.activation(
                    ex[:, 0:W],
                    xs[:, 0:W],
                    mybir.ActivationFunctionType.Exp,
                    bias=nmx[:, :],
                    scale=1.0,
                    accum_out=sm[:, :],
                )
                rs = stats.tile([P, 1], f32, tag="r")
                nc.vector.reciprocal(rs, sm)
                ot = work.tile([P, DV], f32, tag="o")
                # normalize alternately on ScalarE/VectorE to balance both;
                # store from SyncE, which is idle during phase 2
                if ti % 2 == 0:
                    nc.scalar.mul(ot[:, 0:W], ex[:, 0:W], rs[:, :])
                else:
                    nc.vector.tensor_scalar_mul(ot[:, 0:W], ex[:, 0:W], rs[:, :])
                nc.sync.dma_start(
                    out=o_d[t * P : (t + 1) * P, 0:W], in_=ot[:, 0:W]
                )

    nc.compile()
    return nc


def _split16_pack(x):
    """[..., n] fp32 -> [..., 2n] fp16 packed [hi | lo] along the last axis."""
    hi = x.astype(np.float16)
    lo = (x - hi.astype(np.float32)).astype(np.float16)
    return np.ascontiguousarray(np.concatenate([hi, lo], axis=-1))


def _part_major(x):
    """[G*128, C] -> [128, G*C]: partition p holds rows p, 128+p, ... so each
    SBUF partition's data is one contiguous DRAM run per block."""
    g = x.shape[0] // P
    return np.ascontiguousarray(
        x.reshape(g, P, x.shape[1]).transpose(1, 0, 2).reshape(P, -1)
    )


def _get_nc(widths):
    key = tuple(widths)
    if key not in _CACHE:
        _install_ntff_hook()
        _CACHE[key] = _build(key)
    return _CACHE[key]


def kernel(K, V, Q, valid_len, _trace=False):
    import ml_dtypes

    from concourse.bass_utils import run_bass_kernel_spmd

    K = np.ascontiguousarray(np.asarray(K, dtype=np.float32))
    V = np.ascontiguousarray(np.asarray(V, dtype=np.float32))
    Q = np.asarray(Q, dtype=np.float32)
    vl = np.asarray(valid_len).astype(np.int64)

    # sort queries by valid_len (row permutation; exact for row-wise softmax)
    perm = np.argsort(vl, kind="stable")
    vls = vl[perm]
    widths = []
    for t in range(QT_TILES):
        w = int(vls[t * P : (t + 1) * P].max()) + 1
        widths.append(min(DV, -(-w // 32) * 32))
    widths = tuple(widths)
    sum_w = sum(widths)

    # Q^T per batch: permuted rows, pre-scaled by 1/sqrt(d), fp16 hi/lo packed
    scale = np.float32(1.0 / math.sqrt(D))
    qp = Q[:, perm, :] * scale
    qt = np.ascontiguousarray(qp.transpose(0, 2, 1))

    # additive mask for the sorted rows, packed per tile: [128, sum_w] bf16
    # (bf16 is exact here: values are only 0 / -1e6-ish; masked lanes
    # underflow to 0 after exp either way, identical to masked_fill)
    col = np.arange(DV, dtype=np.int64)
    mask_full = np.where(
        col[None, :] > vls[:, None], np.float32(NEG_FILL), np.float32(0.0)
    )
    mask_packed = np.empty((P, sum_w), dtype=ml_dtypes.bfloat16)
    off = 0
    for t in range(QT_TILES):
        w = widths[t]
        mask_packed[:, off : off + w] = mask_full[t * P : (t + 1) * P, :w].astype(
            ml_dtypes.bfloat16
        )
        off += w

    nc = _get_nc(widths)
    in_maps = [
        {
            "k": _part_major(_split16_pack(K[b])),
            "v": _part_major(_split16_pack(V[b])),
            "qt": _part_major(_split16_pack(qt[b])),
            "mask": mask_packed,
        }
        for b in range(N_CORES)
    ]
    res = run_bass_kernel_spmd(
        nc, in_maps, core_ids=list(range(N_CORES)), trace=_trace
    )
    # device rows r correspond to original queries perm[r]; unwritten
    # (masked) columns stay 0 from the pre-zeroed output buffers
    out = np.empty((B, SQ, DV), dtype=np.float32)
    for b in range(N_CORES):
        out[b, perm, :] = res.results[b]["o"]
    if _trace:
        kernel.last_result = res
    return out


# revision 37
# speedup vs baseline: 1.0483x; 1.0066x over previous
"""Trainium2 Bass kernel for nn_DotAtt_40097814675537.

Math (matches the reference exactly up to fp rounding):
    score = Q @ K^T / sqrt(d)        [B, Sq, Sk]
    x     = score @ V                [B, Sq, dv]
    out   = softmax(where(j > valid_len[q], -1e6, x[b, q, j]), axis=-1)

Optimizations:
  * Associativity: x = (Q / sqrt(d)) @ (K^T @ V) - 4x fewer FLOPs
    (contraction 2048 -> 512 for the big matmul; no nonlinearity between
    the two matmuls so this is exact math, only fp rounding differs).
  * Data-parallel over batch B=8, one batch per NeuronCore, no collectives.
  * fp32-accurate matmuls from 3 float16 passes (hi/lo split): each fp32
    operand x = hi + lo with hi=fp16(x), lo=fp16(x-hi), then
    a@b ~= ah@bh + ah@bl + al@bh (al@bl ~ 2^-22 rel, dropped).  Runs at
    the fp16 PE rate: 3 cyc/row total vs 4 cyc/row for native fp32.
  * Sorted-query specialization: rows whose mask kills column j produce
    EXACTLY 0 in the output (exp underflows), so for each 128-row tile only
    columns [0, max(valid_len)+1) need computing.  The host sorts queries
    by valid_len (softmax is row-wise, so a row permutation is exact) and
    the kernel computes a per-tile column width; unwritten output stays 0
    (output buffers are pre-zeroed).  Host inverse-permutes the result.
    The build is cached per width-tuple, so any input data is handled
    correctly (seed-dependent widths just trigger a rebuild).
"""

import math
import sys
import types

import numpy as np

B, SQ, SK, D, DV = 8, 2048, 2048, 512, 512
N_CORES = 8
P = 128  # partitions
SC = SK // P  # 16 s-chunks for the K^T V contraction
DC = D // P  # 4 d-chunks for the Q M contraction
QT_TILES = SQ // P  # 16 query row tiles
NEG_FILL = -1000000.0

_CACHE = {}


def _install_ntff_hook():
    """antenv.axon_hooks is absent in this image; provide it so trace=True
    profiling works when requested (used by test.py, harmless otherwise)."""
    if "antenv.axon_hooks" in sys.modules:
        return
    try:
        from trn_agent_boot.trn_boot import _ntff_profile_via_ctypes

        hook = _ntff_profile_via_ctypes("/opt/axon/libaxon_pjrt.so")
    except Exception:
        hook = None
    mod = types.ModuleType("antenv.axon_hooks")
    mod.get_axon_ntff_profile_hook = lambda: hook
    mod.set_axon_ntff_profile_hook = lambda h: None
    sys.modules["antenv.axon_hooks"] = mod


def _build(widths):
    import concourse.tile as tile
    from concourse import bacc, mybir

    nc = bacc.Bacc("TRN2", target_bir_lowering=False, debug=False, num_devices=N_CORES)
    f32 = mybir.dt.float32
    f16 = mybir.dt.float16
    bf16 = mybir.dt.bfloat16

    sum_w = sum(widths)
    offs = [0]
    for w in widths:
        offs.append(offs[-1] + w)

    # All big inputs arrive PARTITION-MAJOR so each SBUF partition's data is
    # one contiguous DRAM run (8KB DMA descriptors instead of 2KB rows; the
    # DMA queues are descriptor-rate-bound otherwise).  Layouts (fp16):
    #   k:  [128, SC*2*D]  k[p, s*1024 + :]   = packed row  s*128+p  of K
    #   v:  [128, SC*2*DV] v[p, s*1024 + :]   = packed row  s*128+p  of V
    #   qt: [128, DC*2*SQ] qt[p, c*4096 + :]  = packed row  c*128+p  of Q^T
    # Each is loaded in NBLK column blocks so compute can start early.
    NBLK = 8
    KCOLS, QCOLS = SC * 2 * D, DC * 2 * SQ
    k_d = nc.dram_tensor("k", [P, KCOLS], f16, kind="ExternalInput")
    v_d = nc.dram_tensor("v", [P, KCOLS], f16, kind="ExternalInput")
    qt_d = nc.dram_tensor("qt", [P, QCOLS], f16, kind="ExternalInput")
    mask_d = nc.dram_tensor("mask", [P, sum_w], bf16, kind="ExternalInput")
    o_d = nc.dram_tensor("o", [SQ, DV], f32, kind="ExternalOutput")

    with tile.TileContext(nc) as tc:
        with (
            tc.tile_pool(name="consts", bufs=1) as consts,
            tc.tile_pool(name="big", bufs=1) as big,
            tc.tile_pool(name="mprime", bufs=1) as mp_pool,
            tc.tile_pool(name="psm", bufs=1, space="PSUM") as psum_m,
            tc.tile_pool(name="psx", bufs=4, space="PSUM") as psum_x,
            tc.tile_pool(name="work", bufs=6) as work,
            tc.tile_pool(name="stats", bufs=8) as stats,
        ):
            mask_t = consts.tile([P, sum_w], bf16, tag="mask")
            kt = big.tile([P, KCOLS], f16, tag="k", name="k_sb")
            vt = big.tile([P, KCOLS], f16, tag="v", name="v_sb")
            qtt = big.tile([P, QCOLS], f16, tag="qt", name="qt_sb")

            # one sequencer issues every load in priority order: the DMA
            # queues drain roughly in issue order, so K/V blocks beat QT/mask.
            # Geometric block sizes (in s-chunks) so the first matmuls start
            # as early as possible while later blocks amortize issue cost.
            CHUNK = 2 * D  # columns per s-chunk
            kblocks = [1, 1, 2, 4, 8]
            pos = 0
            for nchunk in kblocks:
                lo, hi = pos * CHUNK, (pos + nchunk) * CHUNK
                nc.sync.dma_start(out=kt[:, lo:hi], in_=k_d[:, lo:hi])
                nc.sync.dma_start(out=vt[:, lo:hi], in_=v_d[:, lo:hi])
                pos += nchunk
            qb = QCOLS // NBLK
            for blk in range(NBLK):
                nc.sync.dma_start(
                    out=qtt[:, blk * qb : (blk + 1) * qb],
                    in_=qt_d[:, blk * qb : (blk + 1) * qb],
                )
            nc.sync.dma_start(out=mask_t, in_=mask_d[:, :])

            # Phase 1: M = K^T V over 16 s-chunks, 3 fp16 passes each
            psums = [
                psum_m.tile([P, DV], f32, tag=f"m{c}", name=f"psum_m{c}")
                for c in range(DC)
            ]
            for s in range(SC):
                base = s * 2 * D
                vh = vt[:, base : base + DV]
                vlo = vt[:, base + DV : base + 2 * DV]
                for c in range(DC):
                    kh = kt[:, base + c * P : base + (c + 1) * P]
                    klo = kt[:, base + D + c * P : base + D + (c + 1) * P]
                    # same-weight passes adjacent to reuse loaded weights
                    nc.tensor.matmul(
                        psums[c][:, :], kh, vh, start=(s == 0), stop=False
                    )
                    nc.tensor.matmul(psums[c][:, :], kh, vlo, start=False, stop=False)
                    nc.tensor.matmul(
                        psums[c][:, :], klo, vh, start=False, stop=(s == SC - 1)
                    )

            # M PSUM -> SBUF split into fp16 hi/lo (ScalarE cast + DVE residual)
            mhis, mlos = [], []
            for c in range(DC):
                mhi = mp_pool.tile([P, DV], f16, tag=f"mh{c}", name=f"mhi{c}")
                nc.scalar.copy(mhi[:, :], psums[c][:, :])
                mlo = mp_pool.tile([P, DV], f16, tag=f"ml{c}", name=f"mlo{c}")
                nc.vector.tensor_sub(mlo[:, :], psums[c][:, :], mhi[:, :])
                mhis.append(mhi)
                mlos.append(mlo)

            # Phase 2: per query tile (width W): X = Q M, mask, softmax, store.
            # Widest tiles first so the last tile's softmax tail is shortest.
            order = sorted(range(QT_TILES), key=lambda i: widths[i], reverse=True)
            for ti, t in enumerate(order):
                W = widths[t]
                px = psum_x.tile([P, DV], f32, tag="x")
                for c in range(DC):
                    qh = qtt[:, c * 2 * SQ + t * P : c * 2 * SQ + (t + 1) * P]
                    qlo = qtt[
                        :, c * 2 * SQ + SQ + t * P : c * 2 * SQ + SQ + (t + 1) * P
                    ]
                    nc.tensor.matmul(
                        px[:, 0:W], qh, mhis[c][:, 0:W], start=(c == 0), stop=False
                    )
                    nc.tensor.matmul(
                        px[:, 0:W], qh, mlos[c][:, 0:W], start=False, stop=False
                    )
                    nc.tensor.matmul(
                        px[:, 0:W],
                        qlo,
                        mhis[c][:, 0:W],
                        start=False,
                        stop=(c == DC - 1),
                    )
                xs = work.tile([P, DV], f32, tag="x")
                nc.vector.tensor_add(
                    xs[:, 0:W], px[:, 0:W], mask_t[:, offs[t] : offs[t] + W]
                )
                nmx = stats.tile([P, 1], f32, tag="nmx")
                nc.vector.tensor_reduce(
                    out=nmx,
                    in_=xs[:, 0:W],
                    axis=mybir.AxisListType.X,
                    op=mybir.AluOpType.max,
                    negate=True,
                )
                ex = work.tile([P, DV], f32, tag="e")
                sm = stats.tile([P, 1], f32, tag="sum")
                nc.scalar.activation(
                    ex[:, 0:W],
                    xs[:, 0:W],
                    mybir.ActivationFunctionType.Exp,
                    bias=nmx[:, :],
                    scale=1.0,
                    accum_out=sm[:, :],
                )
                rs = stats.tile([P, 1], f32, tag="r")
                nc.vector.reciprocal(rs, sm)
                ot = work.tile([P, DV], f32, tag="o")
                # normalize alternately on ScalarE/VectorE to balance both;
                # store from SyncE, which is idle during phase 2
                if ti % 2 == 0:
                    nc.scalar.mul(ot[:, 0:W], ex[:, 0:W], rs[:, :])
                else:
                    nc.vector.tensor_scalar_mul(ot[:, 0:W], ex[:, 0:W], rs[:, :])
                nc.sync.dma_start(
                    out=o_d[t * P : (t + 1) * P, 0:W], in_=ot[:, 0:W]
                )

    nc.compile()
    return nc


def _split16_pack(x):
    """[..., n] fp32 -> [..., 2n] fp16 packed [hi | lo] along the last axis."""
    hi = x.astype(np.float16)
    lo = (x - hi.astype(np.float32)).astype(np.float16)
    return np.ascontiguousarray(np.concatenate([hi, lo], axis=-1))


def _part_major(x):
    """[G*128, C] -> [128, G*C]: partition p holds rows p, 128+p, ... so each
    SBUF partition's data is one contiguous DRAM run per block."""
    g = x.shape[0] // P
    return np.ascontiguousarray(
        x.reshape(g, P, x.shape[1]).transpose(1, 0, 2).reshape(P, -1)
    )


def _get_nc(widths):
    key = tuple(widths)
    if key not in _CACHE:
        _install_ntff_hook()
        _CACHE[key] = _build(key)
    return _CACHE[key]


def kernel(K, V, Q, valid_len, _trace=False):
    import ml_dtypes

    from concourse.bass_utils import run_bass_kernel_spmd

    K = np.ascontiguousarray(np.asarray(K, dtype=np.float32))
    V = np.ascontiguousarray(np.asarray(V, dtype=np.float32))
    Q = np.asarray(Q, dtype=np.float32)
    vl = np.asarray(valid_len).astype(np.int64)

    # sort queries by valid_len (row permutation; exact for row-wise softmax)
    perm = np.argsort(vl, kind="stable")
    vls = vl[perm]
    widths = []
    for t in range(QT_TILES):
        w = int(vls[t * P : (t + 1) * P].max()) + 1
        widths.append(min(DV, -(-w // 32) * 32))
    widths = tuple(widths)
    sum_w = sum(widths)

    # Q^T per batch: permuted rows, pre-scaled by 1/sqrt(d), fp16 hi/lo packed
    scale = np.float32(1.0 / math.sqrt(D))
    qp = Q[:, perm, :] * scale
    qt = np.ascontiguousarray(qp.transpose(0, 2, 1))

    # additive mask for the sorted rows, packed per tile: [128, sum_w] bf16
    # (bf16 is exact here: values are only 0 / -1e6-ish; masked lanes
    # underflow to 0 after exp either way, identical to masked_fill)
    col = np.arange(DV, dtype=np.int64)
    mask_full = np.where(
        col[None, :] > vls[:, None], np.float32(NEG_FILL), np.float32(0.0)
    )
    mask_packed = np.empty((P, sum_w), dtype=ml_dtypes.bfloat16)
    off = 0
    for t in range(QT_TILES):
        w = widths[t]
        mask_packed[:, off : off + w] = mask_full[t * P : (t + 1) * P, :w].astype(
            ml_dtypes.bfloat16
        )
        off += w

    nc = _get_nc(widths)
    in_maps = [
        {
            "k": _part_major(_split16_pack(K[b])),
            "v": _part_major(_split16_pack(V[b])),
            "qt": _part_major(_split16_pack(qt[b])),
            "mask": mask_packed,
        }
        for b in range(N_CORES)
    ]
    res = run_bass_kernel_spmd(
        nc, in_maps, core_ids=list(range(N_CORES)), trace=_trace
    )
    # device rows r correspond to original queries perm[r]; unwritten
    # (masked) columns stay 0 from the pre-zeroed output buffers
    out = np.empty((B, SQ, DV), dtype=np.float32)
    for b in range(N_CORES):
        out[b, perm, :] = res.results[b]["o"]
    if _trace:
        kernel.last_result = res
    return out
